# revision 52
# baseline (speedup 1.0000x reference)
"""Trainium2 Bass kernel for the BaseMemory coref scoring module.

Computes, for full inputs (M=65536 memory slots, D=768, E=20, H=64):
    score = relu(pair @ W1 + b1) @ W2 + b2, masked with ent_counter>0,
    where pair = [mem, ment, mem*ment, dist_emb, cnt_emb].

Sharding: data-parallel over the cluster dimension M across 8 NeuronCores.

Device work is a single streamed bf16 matmul pipeline; everything cheap is
folded on the host (O(M) / O(D*H) work):
  - mem@W1_mem + (mem*ment)@W1_had = mem @ (W1_mem + diag(ment)@W1_had)
  - ment@W1_ment + b1 folded into the 10-row dist bucket table
  - bucket embedding lookups precomputed as a [21, M] one-hot/mask matrix,
    contracted on the PE against the folded bucket tables (exact)
  - mem shard pre-cast to bf16, laid out so each DMA is one contiguous 1MB
    read delivering one chunk-pair across 4 row-groups
  - PE schedule batches 4 same-weight matmuls per LDWEIGHTS; scores for two
    groups share one matmul via a block-diagonal W2; the mask/bias terms are
    a host-precomputed row added by the vector engine on the way out
"""

import os
import numpy as np
from ml_dtypes import bfloat16

_jp = os.environ.get("JAX_PLATFORMS")
if _jp is not None and _jp != "" and "axon" not in _jp:
    os.environ["JAX_PLATFORMS"] = "axon," + _jp

M, D, E, H = 65536, 768, 20, 64
N_CORES = 8
MS = M // N_CORES          # rows per core = 8192
GROUP = 512                # rows per PE matmul
N_GROUPS = MS // GROUP     # 16
SG = 4                     # groups per supergroup (weight-batch unit)
N_SG = N_GROUPS // SG      # 4
KCH = D // 128             # 6 contraction chunks
KP = KCH // 2              # 3 chunk-pairs (1MB DMA units)
NF = 21                    # masked flag + 10 dist onehot + 10 cnt onehot
BIG = float(2 ** 20)       # pre-relu kill value for masked rows

_CACHE = {}


def _build():
    """Build + compile the 8-core SPMD bass program once per process."""
    if "nc" in _CACHE:
        return _CACHE["nc"]

    import concourse.bass as bass
    import concourse.mybir as mybir
    import concourse.tile as tile
    from concourse import bacc

    F32 = mybir.dt.float32
    BF16 = mybir.dt.bfloat16

    nc = bacc.Bacc("TRN2", target_bir_lowering=False, debug=False,
                   enable_asserts=False, num_devices=N_CORES)

    xt_d = nc.dram_tensor("xt", [N_SG, KP, 128, 2, SG * GROUP], BF16,
                          kind="ExternalInput").ap()
    f_d = nc.dram_tensor("feat", [NF, MS], BF16, kind="ExternalInput").ap()
    w1_d = nc.dram_tensor("w1", [128, KCH, H], BF16,
                          kind="ExternalInput").ap()
    tcat_d = nc.dram_tensor("tcat", [NF, H], BF16, kind="ExternalInput").ap()
    wsc_d = nc.dram_tensor("wsc", [128, 2], BF16, kind="ExternalInput").ap()
    out_d = nc.dram_tensor("out", [MS], F32, kind="ExternalOutput").ap()
    # the last supergroup ships its relu activations instead of scores; the
    # tiny [2048,64]@[64] score matvec happens on the host, cutting the
    # score->copy->out chain off the critical tail (one combined transfer:
    # two would serialize their issue+transfer latencies)
    hto_d = nc.dram_tensor("hto", [128, 2, GROUP], BF16,
                           kind="ExternalOutput").ap()

    f_r = f_d.rearrange("f (g c) -> f g c", g=N_GROUPS)
    # m = 2048*s + 1024*jj + 512*r + c  ->  [s][r, jj, c]
    out_r = out_d.rearrange("(s jj r c) -> s r jj c", jj=2, r=2, c=GROUP)

    relu = mybir.ActivationFunctionType.Relu

    with tile.TileContext(nc) as tc:
        with (
            tc.tile_pool(name="consts", bufs=1) as cpool,
            tc.tile_pool(name="xin", bufs=N_SG * KP) as px,
            tc.tile_pool(name="ht", bufs=4) as pht,
            tc.tile_pool(name="osb", bufs=2) as posb,
            tc.tile_pool(name="psz", bufs=6, space="PSUM") as psz,
            tc.tile_pool(name="pss", bufs=2, space="PSUM") as pss,
        ):
            # consts go on the scalar HWDGE queue so the big xt DMAs (sync
            # queue) start immediately; ordered so the first supergroup's
            # dependencies (tcat table + its fall slice, then w1) land first
            # even though this ring shares SDMA engines with the xt stream
            tcat = cpool.tile([NF, H], BF16, tag="tcat")
            nc.scalar.dma_start(tcat[:], tcat_d[:])
            fall = cpool.tile([NF, N_GROUPS, GROUP], BF16, tag="fall")
            nc.scalar.dma_start(fall[:, 0:SG, :], f_r[:, 0:SG, :])
            w1t = cpool.tile([128, KCH, H], BF16, tag="w1t")
            nc.scalar.dma_start(w1t[:], w1_d[:])
            nc.scalar.dma_start(fall[:, SG:N_GROUPS, :], f_r[:, SG:N_GROUPS, :])
            wsc2 = cpool.tile([128, 2], BF16, tag="wsc2")
            nc.scalar.dma_start(wsc2[:], wsc_d[:])

            # stream the whole shard: 1MB contiguous reads on the sync ring
            # (the consumer sees each entry's completion ~1-2µs after its
            # data, so entries must stay fine-grained; a second ring only
            # disrupts the HBM access pattern); the last supergroup is split
            # into per-chunk 512KB slices so little work remains after the
            # final byte
            xts = []
            for s in range(N_SG):
                row = []
                for kp in range(KP):
                    xk = px.tile([128, 2, SG * GROUP], BF16, tag="xin")
                    if s == N_SG - 1:
                        nc.sync.dma_start(xk[:, 0, :], xt_d[s, kp, :, 0, :])
                        nc.sync.dma_start(xk[:, 1, :], xt_d[s, kp, :, 1, :])
                    else:
                        nc.sync.dma_start(xk[:], xt_d[s, kp])
                    row.append(xk)
                xts.append(row)

            def emit_scores(s, hts):
                # raw scores only; the mask/bias row is added on the host
                osb2 = posb.tile([2, 2, GROUP], F32, tag="osb")
                for jj in range(2):
                    sc2 = pss.tile([2, GROUP], F32, tag="pss")
                    nc.tensor.matmul(sc2[:], wsc2[:], hts[jj][:],
                                     start=True, stop=True)
                    if jj == 0:
                        nc.scalar.copy(osb2[:, 0, :], sc2[:])
                    else:
                        nc.vector.tensor_copy(osb2[:, 1, :], sc2[:])
                # HWDGE out on the scalar ring: empty after the small consts,
                # so outputs never queue behind the input stream and the
                # final one issues with minimal latency
                nc.scalar.dma_start(out_r[s], osb2[:])

            pending = None
            for s in range(N_SG):
                zts = []
                for j in range(SG):
                    zt = psz.tile([H, GROUP], F32, tag="psz")
                    zts.append(zt)
                # k0/k1 passes lead (their data semaphore is the earliest
                # ready), the feature pass follows so nothing but relu
                # remains on the critical path after the last x chunk
                for kp in range(KP):
                    xk = xts[s][kp]
                    for kk in range(2):
                        k = 2 * kp + kk
                        for j in range(SG):
                            nc.tensor.matmul(
                                zts[j][:], w1t[:, k, :],
                                xk[:, kk, GROUP * j:GROUP * (j + 1)],
                                start=(k == 0), stop=(k == KCH - 1))
                    if kp == 0:
                        if pending is not None:
                            # previous supergroup's scores: its relus
                            # overlapped this supergroup's first passes
                            emit_scores(*pending)
                            pending = None
                        for j in range(SG):
                            nc.tensor.matmul(zts[j][:], tcat[:],
                                             fall[:, SG * s + j, :],
                                             start=False, stop=False)
                if s < N_SG - 1:
                    hts = []
                    for jj in range(2):
                        htp = pht.tile([128, GROUP], BF16, tag="ht")
                        hts.append(htp)
                    for j in range(SG):
                        dst = hts[j // 2][H * (j % 2):H * (j % 2 + 1), :]
                        if j % 2 == 0:
                            nc.scalar.activation(dst, zts[j][:], relu)
                        else:
                            nc.vector.tensor_scalar_max(dst, zts[j][:], 0.0)
                    pending = (s, hts)
                else:
                    htb = pht.tile([128, 2, GROUP], BF16, tag="htb")
                    for j in range(SG):
                        dst = htb[H * (j % 2):H * (j % 2 + 1), j // 2, :]
                        if j % 2 == 0:
                            nc.scalar.activation(dst, zts[j][:], relu)
                        else:
                            nc.vector.tensor_scalar_max(dst, zts[j][:], 0.0)
                    # sync ring is idle once the input stream ends; this
                    # issues without coupling to the ACT instruction queue
                    nc.sync.dma_start(hto_d[:], htb[:])

    nc.compile()
    _CACHE["nc"] = nc
    return nc


def _get_bucket(c):
    """Identity buckets for c<=4, floor(log2) buckets above, clamped to
    [0, 9]. Integer-exact; matches the f32 jax reference on [0, 2^20]."""
    c = np.asarray(c).astype(np.int64)
    cl = np.maximum(c, 1)
    fl = np.frexp(cl.astype(np.float64))[1] - 1   # floor(log2), exact
    idx = np.where(c <= 4, c, fl + 3)
    return np.clip(idx, 0, 9).astype(np.int64)


def _prepare_maps(ment_emb, mem_vectors, dist_table, counter_table,
                  W1, b1, W2, b2, ent_counter, last_mention_start, ment_start):
    f32 = np.float32
    ment = np.asarray(ment_emb, f32)
    mem = np.asarray(mem_vectors, f32)
    W1 = np.asarray(W1, f32)
    ms_i = int(np.asarray(ment_start))

    W1m, W1r, W1h = W1[0:D], W1[D:2 * D], W1[2 * D:3 * D]
    W1d, W1c = W1[3 * D:3 * D + E], W1[3 * D + E:3 * D + 2 * E]

    w1eff = (W1m + ment[:, None] * W1h).astype(f32)              # [768, 64]
    bias_vec = (np.asarray(b1, f32) + ment @ W1r).astype(f32)    # [64]
    T_d = (np.asarray(dist_table, f32) @ W1d + bias_vec).astype(f32)
    T_c = (np.asarray(counter_table, f32) @ W1c).astype(f32)
    b2v = float(np.asarray(b2, f32).reshape(-1)[0])

    # feature rows: [masked, dist onehots, cnt onehots]; masked row kills
    # z pre-relu so masked scores come exactly from the fc row below
    tcat = np.concatenate(
        [np.full((1, H), -BIG, f32), T_d, T_c], 0).astype(bfloat16)
    wsc2 = np.zeros((128, 2), f32)
    wsc2[0:H, 0] = np.asarray(W2, f32).reshape(-1)
    wsc2[H:2 * H, 1] = np.asarray(W2, f32).reshape(-1)
    wsc2 = wsc2.astype(bfloat16)

    w1t = np.ascontiguousarray(
        w1eff.reshape(KCH, 128, H).transpose(1, 0, 2)).astype(bfloat16)

    cnt = np.asarray(ent_counter).astype(np.int64)
    lms = np.asarray(last_mention_start).astype(np.int64)
    bd = _get_bucket(ms_i - lms)
    bc = _get_bucket(cnt)
    F = np.zeros((NF, M), f32)
    rows = np.arange(M)
    masked = (cnt <= 0)
    F[0] = masked
    F[1 + bd, rows] = 1.0
    F[11 + bc, rows] = 1.0
    F = F.astype(bfloat16)

    # device returns raw W2.relu(z); host adds fc = masked*-10000 + b2
    # (masked rows have relu(z) == 0 exactly, so masking is exact)
    fcrow = np.where(masked, np.float32(-10000.0), np.float32(b2v))

    mem16 = mem.astype(bfloat16)

    in_maps = []
    for c in range(N_CORES):
        sl = slice(c * MS, (c + 1) * MS)
        # [m, d] -> [s, kp, p, kk, j, c]; m = 2048s+512j+c, d = 256kp+128kk+p
        xt = np.ascontiguousarray(
            mem16[sl].reshape(N_SG, SG, GROUP, KP, 2, 128)
            .transpose(0, 3, 5, 4, 1, 2).reshape(N_SG, KP, 128, 2, SG * GROUP))
        in_maps.append(dict(
            xt=xt, feat=np.ascontiguousarray(F[:, sl]),
            w1=w1t, tcat=tcat, wsc=wsc2))
    return in_maps, (fcrow, np.asarray(W2, f32).reshape(-1))


def _postprocess(results, post):
    fcrow, w2f = post
    out = np.empty(M + 1, np.float32)
    # device m index = 2048s + 1024jj + 512r + c (identity permutation of m)
    dev_rows = (N_SG - 1) * SG * GROUP
    for c in range(N_CORES):
        base = c * MS
        out[base:base + dev_rows] = results[c]["out"][:dev_rows]
        # last supergroup: scores from the shipped relu activations
        hto = np.asarray(results[c]["hto"], np.float32)  # [128, 2, GROUP]
        for jj in range(2):
            for r in range(2):
                g = (N_SG - 1) * SG + 2 * jj + r
                out[base + g * GROUP:base + (g + 1) * GROUP] = \
                    w2f @ hto[H * r:H * (r + 1), jj]
    out[:M] += fcrow
    out[M] = 0.0
    return out


def run_spmd(in_maps, trace=False):
    from concourse.bass_utils import run_bass_kernel_spmd
    nc = _build()
    return run_bass_kernel_spmd(nc, in_maps, list(range(N_CORES)), trace=trace)


def kernel(**inputs):
    in_maps, fcrow = _prepare_maps(**inputs)
    res = run_spmd(in_maps, trace=False)
    return _postprocess(res.results, fcrow)
